# revision 23
# baseline (speedup 1.0000x reference)
"""Trainium2 Bass kernel for nn_Decoder (causal CNN-GLU decoder with attention).

Computation (per batch):
  x  = shift_right(mel @ W_lin.T + b_lin)
  h1 = causal_cnn_glu(x, w0, b0)              # k=5, D->2D, GLU, residual, /sqrt2
  q  = h1 @ W_attn.T + b_attn
  A  = softmax(q @ enc.T) ; c = A @ (enc + femb)
  h2 = causal_cnn_glu(h1 + c, w1, b1)
  out = h2 @ W_proj.T + b_proj

Sharding: data-parallel over batch B=32 across 8 cores (4 batches/core),
weights replicated.  All activations on-chip are kept feature-major
([D partitions, T free]) so the causal conv taps are just shifted slices
along the free dim and matmul contractions stay on the partition dim.

Scale folding: the two /sqrt(2) are folded into the weights so the GLU
epilogue is exactly two DVE ops per half:
  x' = x/sqrt2  (W_lin,b_lin scaled), conv g-halves scaled by sqrt2,
  conv a-biases scaled by 1/sqrt2, attention context scaled by
  1/(sqrt2*denom) during normalization.
"""

import sys

try:  # prefer the environment's concourse (axon site); fall back to /opt
    import concourse  # noqa: F401
except ImportError:
    sys.path.insert(0, "/opt/trn_rl_repo")

from contextlib import ExitStack  # noqa: E402

import numpy as np  # noqa: E402

import concourse.bass as bass  # noqa: E402
import concourse.mybir as mybir  # noqa: E402
import concourse.tile as tile  # noqa: E402
from concourse import bacc  # noqa: E402
from concourse.masks import make_identity  # noqa: E402

F32 = mybir.dt.float32
F32R = mybir.dt.float32r
BF16 = mybir.dt.bfloat16
AF = mybir.ActivationFunctionType
OP = mybir.AluOpType

B, T_ENC, T_DEC, D, IN = 32, 1024, 2048, 256, 80
NCORES = 8
BPC = B // NCORES
SQRT2 = float(np.sqrt(2.0))
ISQ2 = float(1.0 / np.sqrt(2.0))
SHIFT = 50.0  # softmax stabilization: probs = exp(score - SHIFT)


def _r(ap):
    return ap.bitcast(F32R)


def build_nc(bpc=BPC, t_enc=T_ENC, t_dec=T_DEC, ch=512, num_devices=NCORES,
             loop_n=1, only_phase=None, pb_mode="pool", no_denom=False,
             rotate=False, tr_ident="f32r"):
    nte = t_enc // 128   # encoder token tiles
    ntd = t_dec // 128   # decoder token tiles
    nch = t_dec // ch    # chunks per batch
    cpt = ch // 128      # 128-token tiles per chunk

    nc = bacc.Bacc("TRN2", target_bir_lowering=False, debug=False,
                   num_devices=num_devices)

    enc_d = nc.dram_tensor("enc", [bpc, t_enc, D], F32R, kind="ExternalInput")
    femb_d = nc.dram_tensor("femb", [bpc, t_enc, D], F32R, kind="ExternalInput")
    mel_d = nc.dram_tensor("mel", [bpc, t_dec, IN], F32R, kind="ExternalInput")
    wlin_d = nc.dram_tensor("wlin", [IN, D], F32R, kind="ExternalInput")
    w0_d = nc.dram_tensor("w0", [128, 5 * 2 * 2 * D], F32R, kind="ExternalInput")
    w1_d = nc.dram_tensor("w1", [128, 5 * 2 * 2 * D], F32R, kind="ExternalInput")
    wattn_d = nc.dram_tensor("wattn", [128, 2 * D], F32R, kind="ExternalInput")
    wproj_d = nc.dram_tensor("wproj", [128, 2 * IN], F32R, kind="ExternalInput")
    bias_d = nc.dram_tensor("bias", [128, 13], F32, kind="ExternalInput")
    ident_d = nc.dram_tensor("ident", [128, 128], F32, kind="ExternalInput")
    out_d = nc.dram_tensor("out", [bpc, t_dec, IN], F32, kind="ExternalOutput")

    with tile.TileContext(nc) as tc, ExitStack() as ctx:
        cpool = ctx.enter_context(tc.tile_pool(name="const", bufs=1))
        stage = ctx.enter_context(tc.tile_pool(name="stage", bufs=4))
        pb = ctx.enter_context(tc.tile_pool(name="perbatch", bufs=1))
        sc = ctx.enter_context(tc.tile_pool(name="scratch", bufs=1))
        sc2 = ctx.enter_context(tc.tile_pool(name="scratch2", bufs=2))
        pmm = ctx.enter_context(
            tc.tile_pool(name="pmm", bufs=6, space=bass.MemorySpace.PSUM))
        pctx = ctx.enter_context(
            tc.tile_pool(name="pctx", bufs=1, space=bass.MemorySpace.PSUM))

        # ---- constants ----
        ident = cpool.tile([128, 128], F32, tag="ident")
        nc.sync.dma_start(out=ident[:], in_=ident_d[:])
        # PE transpose cost is keyed on the ifmap (= identity) dtype:
        # f32=2.0, f32r=1.5 cycles/row.  walrus requires both matmul operand
        # transfer types to match when either is f32/f32r, so the data is
        # bitcast to f32r alongside (values pass through the PE exactly).
        if tr_ident == "f32r":
            identT = cpool.tile([128, 128], F32R, tag="identT")
            nc.vector.tensor_copy(identT[:], ident[:])
        else:
            identT = ident
        # ones column scaled by sqrt2: denominator comes out as sqrt2*sum(p),
        # so reciprocal directly gives isq2/sum(p) (folds the GLU 1/sqrt2).
        # (DVE memset can't write f32r; stage in f32 and copy with rounding.)
        ones_f32 = cpool.tile([128, 1], F32, tag="ones_f32")
        nc.vector.memset(ones_f32[:], SQRT2)
        ones_col = cpool.tile([128, 1], F32R, tag="ones")
        nc.vector.tensor_copy(ones_col[:], ones_f32[:])
        onesr_f32 = cpool.tile([1, 128], F32, tag="onesr_f32")
        nc.vector.memset(onesr_f32[:], 1.0)
        ones_row = cpool.tile([1, 128], F32R, tag="ones_row")
        nc.vector.tensor_copy(ones_row[:], onesr_f32[:])
        zero4 = cpool.tile([128, 4], F32, tag="zero4")
        nc.vector.memset(zero4[:], 0.0)
        negshift = cpool.tile([128, 1], F32, tag="negshift")
        nc.vector.memset(negshift[:], -SHIFT)

        wlin = cpool.tile([IN, D], F32R, tag="wlin")
        w0 = cpool.tile([128, 5 * 2 * 2 * D], F32R, tag="w0")
        w1 = cpool.tile([128, 5 * 2 * 2 * D], F32R, tag="w1")
        wattn = cpool.tile([128, 2 * D], F32R, tag="wattn")
        wproj = cpool.tile([128, 2 * IN], F32R, tag="wproj")
        bias = cpool.tile([128, 13], F32, tag="bias")

        def load_weights_early():
            nc.sync.dma_start(out=wlin[:], in_=wlin_d[:])
            nc.sync.dma_start(out=bias[:], in_=bias_d[:])

        def load_weights_mid():
            nc.sync.dma_start(out=w0[:], in_=w0_d[:])

        def load_weights_late():
            nc.sync.dma_start(out=wattn[:], in_=wattn_d[:])
            nc.sync.dma_start(out=wproj[:], in_=wproj_d[:])
            nc.sync.dma_start(out=w1[:], in_=w1_d[:])

        def bcol(j):
            return bias[:, j:j + 1]

        def tr(out_ap, in_ap, id_ap):
            # f32 data transposed under a non-f32 identity must be bitcast to
            # f32r (values pass through the PE exactly; only dtype tags change)
            if tr_ident == "f32":
                nc.tensor.transpose(out_ap, in_ap, id_ap)
            else:
                nc.tensor.transpose(out_ap.bitcast(F32R), in_ap.bitcast(F32R),
                                    id_ap)

        def conv_glu(w_sb, ba0, bg0, in_buf, base, out_ap_fn, resid_ap_fn):
            """One causal-conv+GLU chunk.  in_buf: [128, 2, T+4] padded buffer.
            out_ap_fn(i) / resid_ap_fn(i) give [128, ch] APs for d-tile i."""
            s_tiles = {}
            for j in (2, 3, 0, 1):
                pc = pmm.tile([128, ch], F32, tag="mm")
                k = 0
                for t in range(5):
                    for i in range(2):
                        col = (t * 2 + i) * (2 * D) + j * 128
                        nc.tensor.matmul(
                            pc[:],
                            _r(w_sb[:, col:col + 128]),
                            _r(in_buf[:, i, base + t:base + t + ch]),
                            start=(k == 0), stop=(k == 9))
                        k += 1
                if j >= 2:
                    s = sc2.tile([128, ch], F32, tag=f"sig{j - 2}", name=f"sig{j - 2}")
                    nc.scalar.activation(s[:], pc[:], AF.Sigmoid,
                                         bias=bcol(bg0 + (j - 2)))
                    s_tiles[j - 2] = s
                else:
                    o = out_ap_fn(j)
                    nc.vector.scalar_tensor_tensor(
                        o, pc[:], bcol(ba0 + j), s_tiles[j][:],
                        op0=OP.add, op1=OP.mult)
                    nc.vector.tensor_add(o, o, resid_ap_fn(j))

        def prep_mel(b, melT, x_buf):
            """mel load (one DMA), transpose to feature-major, zero pads."""
            with nc.named_scope(f"prepmel{b}"):
                if b == 0:
                    load_weights_early()
                nc.vector.tensor_copy(melT[:, 0:1], zero4[0:IN, 0:1])
                for i in range(2):
                    nc.vector.tensor_copy(x_buf[:, i, 0:4], zero4[:])
                mt = stage.tile([128, ntd, IN], F32R, tag="mt", name="mt", bufs=1)
                mview = mel_d[b].rearrange("(n p) f -> p n f", p=128)
                # quarter the load so the first transpose starts ~4x earlier
                nq = ntd // 4
                for qq in range(4):
                    nc.sync.dma_start(out=mt[:, qq * nq:(qq + 1) * nq, :],
                                      in_=mview[:, qq * nq:(qq + 1) * nq, :])
                for n in range(ntd):
                    pt = pmm.tile([IN, 128], F32, tag="mm", name="pt")
                    tr(pt[:], mt[:, n, :], identT[:])
                    dst = melT[:, 1 + n * 128:1 + (n + 1) * 128]
                    if n % 2 == 0:
                        nc.vector.tensor_copy(dst, pt[:])
                    else:
                        nc.scalar.copy(dst, pt[:])
                if b == 0:
                    load_weights_mid()

        def prep_enc(b, encT, encsum, eT, bias_phB, battn_r):
            # enc/femb are loaded with partition-outer token order (contiguous
            # 8KB runs per partition, ~8x fewer DMA descriptors).  This
            # permutes encoder positions, which cancels: encT (scores
            # stationary) and encsum (context stationary) use the identical
            # permutation and attention is permutation-invariant over ts.
            with nc.named_scope(f"prepenc{b}"):
                et = stage.tile([128, nte, D], F32R, tag="et", name="et", bufs=1)
                nc.sync.dma_start(
                    out=et[:], in_=enc_d[b].rearrange("(p n) d -> p n d", p=128))
                for n in range(nte):
                    for i in range(2):
                        pt = pmm.tile([128, 128], F32, tag="mm", name="pt")
                        tr(pt[:], et[:, n, i * 128:(i + 1) * 128], identT[:])
                        nc.scalar.copy(encT[:, i, n * 128:(n + 1) * 128], pt[:])
                if b == 0:
                    load_weights_late()
                    for i2 in range(2):
                        nc.vector.tensor_copy(battn_r[:, i2, 0:1],
                                              bias[:, 10 + i2:11 + i2])
                        nc.vector.tensor_copy(battn_r[:, i2, 1:2],
                                              zero4[:, 0:1])
                # ẽ = W_attn^T·enc in feature-major [d̃, s]: replaces the
                # per-chunk q projection (1024 enc rows vs 2048 dec rows).
                S = min(ch, 512)
                for o in range(2):
                    for sh in range(t_enc // S):
                        pe_t = pmm.tile([128, ch], F32, tag="mm", name="pe_t")
                        for i in range(2):
                            col = (i * 2 + o) * 128
                            nc.tensor.matmul(
                                pe_t[:, 0:S],
                                _r(wattn[:, col:col + 128]),
                                _r(encT[:, i, sh * S:(sh + 1) * S]),
                                start=(i == 0), stop=(i == 1))
                        nc.scalar.copy(eT[:, o, sh * S:(sh + 1) * S],
                                       pe_t[:, 0:S])
                # β_j[s] = b_attn·enc_s folded into the exp bias (with -SHIFT)
                pbeta = pmm.tile([128, ch], F32, tag="mm", name="pbeta")
                for j in range(nte):
                    for i in range(2):
                        nc.tensor.matmul(pbeta[:, 2 * j:2 * j + 2],
                                         _r(encT[:, i, j * 128:(j + 1) * 128]),
                                         battn_r[:, i, :],
                                         start=(i == 0), stop=(i == 1))
                nc.scalar.activation(
                    bias_phB[:],
                    pbeta[:].rearrange("p (h two) -> p h two", two=2)[:, 0:nte, 0],
                    AF.Identity, bias=negshift[:])
                fview = femb_d[b].rearrange("(p n) d -> p n d", p=128)
                for h in range(2):
                    nh = nte // 2
                    ft = stage.tile([128, nh, D], F32R, tag="ft", name="ft",
                                    bufs=2)
                    nc.sync.dma_start(out=ft[:], in_=fview[:, h * nh:(h + 1) * nh, :])
                    for n in range(nh):
                        nc.vector.tensor_add(encsum[:, h * nh + n, :],
                                             et[:, h * nh + n, :], ft[:, n, :])

        def body_emit():
            melTs, x_bufs, h1_bufs = {}, {}, {}

            def alloc_mel(b):
                melTs[b] = pb.tile([IN, t_dec + 1], F32R, tag="melT", name="melT")
                x_bufs[b] = pb.tile([128, 2, t_dec + 4], F32R, tag="x_buf",
                                    name="x_buf")

            def emit_phA(b):
                """linear + conv0 for all chunks of batch b (sigmoid table)."""
                melT = melTs.pop(b)
                x_buf = x_bufs[b]
                h1_bufs[b] = pb.tile([128, 2, t_dec], F32R, tag="h1_buf",
                                     name="h1_buf")
                h1_buf = h1_bufs[b]
                with nc.named_scope(f"phA_{b}"):
                    for c in range(nch):
                        base = c * ch
                        for i in range(2):
                            px = pmm.tile([128, ch], F32, tag="mm", name="px")
                            nc.tensor.matmul(px[:],
                                             _r(wlin[:, i * 128:(i + 1) * 128]),
                                             _r(melT[:, base:base + ch]),
                                             start=True, stop=True)
                            nc.scalar.activation(
                                x_buf[:, i, 4 + base:4 + base + ch],
                                px[:], AF.Identity, bias=bcol(0 + i))
                        if c == 0:
                            # x[0] must be exactly 0 (shift pad), not b_lin
                            for i2 in range(2):
                                nc.vector.tensor_copy(x_buf[:, i2, 4:5],
                                                      zero4[:, 0:1])
                    for c in range(nch):
                        base = c * ch
                        conv_glu(w0, 2, 4, x_buf, base,
                                 lambda i: h1_buf[:, i, base:base + ch],
                                 lambda i: x_buf[:, i, 4 + base:4 + base + ch])

            if only_phase in ("B", "C"):
                # isolated-phase builds still need weights + dummy producers
                load_weights_early()
                load_weights_mid()
                load_weights_late()
            alloc_mel(0)
            if only_phase in (None, "A"):
                prep_mel(0, melTs[0], x_bufs[0])
                emit_phA(0)

            battn_r = cpool.tile([128, 2, 2], F32R, tag="battn_r")
            for b in range(bpc):
                if not rotate and b > 0 and only_phase in (None, "A"):
                    emit_phA(b)
                encT = pb.tile([128, 2, t_enc], F32R, tag="encT", name="encT")
                encsum = pb.tile([128, nte, D], F32R, tag="encsum", name="encsum")
                eT = pb.tile([128, 2, t_enc], F32R, tag="eT", name="eT")
                bias_phB = pb.tile([128, nte], F32, tag="bias_phB",
                                   name="bias_phB")
                hA_buf = pb.tile([128, 2, t_dec + 4], F32R, tag="hA_buf",
                                 name="hA_buf")
                if only_phase == "B":
                    h1_bufs[b] = pb.tile([128, 2, t_dec], F32R, tag="h1_buf",
                                         name="h1_buf")
                h1_buf = h1_bufs.get(b)

                if only_phase in (None, "B"):
                    prep_enc(b, encT, encsum, eT, bias_phB, battn_r)
                # next batch's mel prep hides under phB/phC of this batch
                if b + 1 < bpc:
                    alloc_mel(b + 1)
                    if only_phase in (None, "A"):
                        prep_mel(b + 1, melTs[b + 1], x_bufs[b + 1])

                # hA zero pads (hA_buf slot frees once conv1 of b-1 is done)
                for i in range(2):
                    nc.vector.tensor_copy(hA_buf[:, i, 0:4], zero4[:])
                if only_phase == "B":
                    for i in range(2):
                        nc.vector.tensor_copy(h1_buf[:, i, 0:4], zero4[:])
                if only_phase == "C":
                    for i in range(2):
                        nc.vector.tensor_copy(hA_buf[:, i, 4:8], zero4[:])

                # ---- phase B: attention for all chunks (exp table) ----
                def scores_exp(c):
                    base = c * ch
                    probs = sc.tile([128, nte, ch], F32R, tag="probs",
                                    name="probs", bufs=2)
                    for j in range(nte):
                        ps = pmm.tile([128, ch], F32, tag="mm", name="ps")
                        for i in range(2):
                            nc.tensor.matmul(ps[:],
                                             _r(eT[:, i, j * 128:(j + 1) * 128]),
                                             _r(h1_buf[:, i, base:base + ch]),
                                             start=(i == 0), stop=(i == 1))
                        nc.scalar.activation(probs[:, j, :], ps[:], AF.Exp,
                                             bias=bias_phB[:, j:j + 1])
                    return probs

                def attn_back(c, probs):
                    base = c * ch
                    pc0 = pctx.tile([128, ch], F32, tag="c0", name="pc0")
                    pc1 = pctx.tile([128, ch], F32, tag="c1", name="pc1")
                    pd = pmm.tile([1, ch], F32, tag="mm", name="pd")
                    for j in range(nte):
                        pr = probs[:, j, :]
                        nc.tensor.matmul(pc0[:], encsum[:, j, 0:128], pr,
                                         start=(j == 0), stop=(j == nte - 1))
                        nc.tensor.matmul(pc1[:], encsum[:, j, 128:256], pr,
                                         start=(j == 0), stop=(j == nte - 1))
                        if not no_denom:
                            nc.tensor.matmul(pd[:], ones_col[:], pr,
                                             start=(j == 0), stop=(j == nte - 1))
                    den_r = sc.tile([1, ch], F32, tag="den", name="den")
                    rep = sc.tile([128, ch], F32, tag="rep", name="rep")
                    if no_denom:
                        nc.vector.memset(rep[:], 1.0)
                    elif pb_mode == "pool":
                        nc.vector.reciprocal(den_r[:], pd[:])
                        nc.gpsimd.partition_broadcast(rep[:], den_r[:])
                    else:
                        nc.vector.reciprocal(den_r[:], pd[:])
                        den_rr = sc.tile([1, ch], F32R, tag="denr", name="den_rr")
                        nc.vector.tensor_copy(den_rr[:], den_r[:])
                        prep_ps = pmm.tile([128, ch], F32, tag="mm", name="prep_ps")
                        nc.tensor.matmul(prep_ps[:], ones_row[:], den_rr[:],
                                         start=True, stop=True)
                        nc.vector.tensor_copy(rep[:], prep_ps[:])
                    pcx = [pc0, pc1]
                    for i in range(2):
                        tmp = sc.tile([128, ch], F32, tag=f"tmp{i}", name=f"tmp{i}")
                        nc.vector.tensor_tensor(tmp[:], pcx[i][:], rep[:], op=OP.mult)
                        # hA' = h1/sqrt2 + ctx_unnorm * (isq2/denom)
                        nc.vector.scalar_tensor_tensor(
                            hA_buf[:, i, 4 + base:4 + base + ch],
                            h1_buf[:, i, base:base + ch], ISQ2, tmp[:],
                            op0=OP.mult, op1=OP.add)

                if only_phase in (None, "B"):
                    with nc.named_scope(f"phB_{b}"):
                        # scores(c+1) is emitted before ctx(c): PE chews on the
                        # next chunk's scores while ACT evaluates exp(c), so the
                        # context matmuls never wait on a cold pipeline.
                        probs_c = scores_exp(0)
                        for c in range(nch):
                            probs_next = (scores_exp(c + 1)
                                          if c + 1 < nch else None)
                            attn_back(c, probs_c)
                            probs_c = probs_next

                # rotated schedule: conv0 of batch b+1 is emitted here, between
                # phB(b) and phC(b).  phC(b) depends on phB(b)'s DVE epilogue
                # (hA), so conv0(b+1) gives PE independent work to chew on while
                # that drains; ACT table order stays exp -> sigmoid -> sigmoid.
                if rotate and b + 1 < bpc and only_phase in (None, "A"):
                    emit_phA(b + 1)

                # ---- phase C: conv1 + proj for all chunks (sigmoid table) ----
                # proj(c-1) is emitted after conv1(c) so PE never waits on the
                # GLU DVE epilogue of chunk c before starting useful work.
                def proj_and_out(c, h2, last=False):
                    base = c * ch
                    pp = pmm.tile([IN, ch], F32, tag="mm", name="pp")
                    for kk in range(2):
                        nc.tensor.matmul(pp[:], _r(wproj[:, kk * IN:(kk + 1) * IN]),
                                         _r(h2[kk][:]), start=(kk == 0),
                                         stop=(kk == 1))
                    proj = sc2.tile([IN, ch], F32R, tag="proj", name="proj")
                    nc.scalar.activation(proj[:], pp[:], AF.Identity,
                                         bias=bias[0:IN, 12:13])
                    for k in range(cpt):
                        if last:
                            # conv is finished: use the big pool so all four
                            # transposes fly in parallel during the drain
                            pt = pmm.tile([128, IN], F32, tag="mm", name="pt")
                        else:
                            pt = pctx.tile([128, IN], F32, tag=("c0" if k % 2 == 0
                                                               else "c1"), name="pt")
                        tr(pt[:], proj[:, k * 128:(k + 1) * 128], identT[0:IN, 0:IN])
                        ot = sc2.tile([128, IN], F32, tag="outT", name="ot")
                        if k % 2 == 0:
                            nc.vector.tensor_copy(ot[:], pt[:])
                        else:
                            nc.scalar.copy(ot[:], pt[:])
                        r0 = base + k * 128
                        # alternate HWDGE queues (SP / Activation) so the
                        # descriptor processing of consecutive tiles overlaps
                        eng = nc.sync if k % 2 == 0 else nc.scalar
                        eng.dma_start(out=out_d[b, r0:r0 + 128, :], in_=ot[:])

                if only_phase in (None, "C"):
                    with nc.named_scope(f"phC_{b}"):
                        h2_prev = None
                        for c in range(nch):
                            base = c * ch
                            h2 = [sc2.tile([128, ch], F32R, tag=f"h2_{i}",
                                           name=f"h2_{i}") for i in range(2)]
                            conv_glu(w1, 6, 8, hA_buf, base,
                                     lambda i: h2[i][:],
                                     lambda i: hA_buf[:, i, 4 + base:4 + base + ch])
                            if h2_prev is not None:
                                proj_and_out(c - 1, h2_prev)
                            h2_prev = h2
                        proj_and_out(nch - 1, h2_prev, last=(b == bpc - 1))
        import contextlib
        loop_cm = (tc.For_i(0, loop_n, 1, hint_engines=(mybir.EngineType.PE,))
                   if loop_n > 1 else contextlib.nullcontext())
        with loop_cm:
            body_emit()

    nc.compile()
    return nc


def prep_weights(W_lin, b_lin, conv_w0, conv_b0, conv_w1, conv_b1,
                 W_attn, b_attn, W_proj, b_proj):
    def prep_conv(w):
        ws = w.astype(np.float32).copy()
        ws[D:] *= SQRT2                       # g-half
        # [512, 256, 5] -> [p, t, i, o] -> [128, 5*2*512]
        arr = ws.transpose(1, 2, 0).reshape(2, 128, 5, 2 * D).transpose(1, 2, 0, 3)
        return np.ascontiguousarray(arr.reshape(128, 5 * 2 * 2 * D))

    wlin_h = np.ascontiguousarray(W_lin.T * ISQ2).astype(np.float32)
    # lhsT layout for ẽ = W_attn^T·enc: [e_part, (i,o), d̃] where i is the
    # e-contraction tile and o the output d̃-tile:
    # wattn_h[p, (i*2+o)*128 + c] = W_attn[i*128+p, o*128+c]
    wattn_h = np.ascontiguousarray(
        np.asarray(W_attn, np.float32).reshape(2, 128, 2, 128)
        .transpose(1, 0, 2, 3).reshape(128, 2 * D))
    wproj_h = np.ascontiguousarray(
        W_proj.T.reshape(2, 128, IN).transpose(1, 0, 2).reshape(128, 2 * IN)
    ).astype(np.float32)

    bias_h = np.zeros((128, 13), np.float32)
    bias_h[:, 0] = b_lin[0:128] * ISQ2
    bias_h[:, 1] = b_lin[128:256] * ISQ2
    bias_h[:, 2] = conv_b0[0:128] * ISQ2      # a-half biases scaled
    bias_h[:, 3] = conv_b0[128:256] * ISQ2
    bias_h[:, 4] = conv_b0[256:384]           # g-half biases unscaled
    bias_h[:, 5] = conv_b0[384:512]
    bias_h[:, 6] = conv_b1[0:128] * ISQ2
    bias_h[:, 7] = conv_b1[128:256] * ISQ2
    bias_h[:, 8] = conv_b1[256:384]
    bias_h[:, 9] = conv_b1[384:512]
    bias_h[:, 10] = b_attn[0:128]
    bias_h[:, 11] = b_attn[128:256]
    bias_h[0:IN, 12] = b_proj

    return {
        "wlin": wlin_h, "w0": prep_conv(conv_w0), "w1": prep_conv(conv_w1),
        "wattn": wattn_h, "wproj": wproj_h, "bias": bias_h,
        "ident": np.eye(128, dtype=np.float32),
    }


_NC = None


def _get_nc():
    global _NC
    if _NC is None:
        _NC = build_nc()
    return _NC


def kernel(encoder_outputs, first_embedding, mel_inputs,
           W_lin, b_lin, conv_w0, conv_b0, conv_w1, conv_b1,
           W_attn, b_attn, W_proj, b_proj):
    from concourse.bass_utils import run_bass_kernel_spmd

    nc = _get_nc()
    w = prep_weights(W_lin, b_lin, conv_w0, conv_b0, conv_w1, conv_b1,
                     W_attn, b_attn, W_proj, b_proj)
    enc = np.asarray(encoder_outputs, np.float32)
    femb = np.asarray(first_embedding, np.float32)
    mel = np.asarray(mel_inputs, np.float32)
    in_maps = []
    for c in range(NCORES):
        sl = slice(c * BPC, (c + 1) * BPC)
        in_maps.append({"enc": np.ascontiguousarray(enc[sl]),
                        "femb": np.ascontiguousarray(femb[sl]),
                        "mel": np.ascontiguousarray(mel[sl]), **w})
    res = run_bass_kernel_spmd(nc, in_maps, list(range(NCORES)))
    return np.concatenate([res.results[i]["out"] for i in range(NCORES)], axis=0)



# revision 28
# speedup vs baseline: 1.1467x; 1.1467x over previous
"""Trainium2 Bass kernel for nn_Decoder (causal CNN-GLU decoder with attention).

Computation (per batch):
  x  = shift_right(mel @ W_lin.T + b_lin)
  h1 = causal_cnn_glu(x, w0, b0)              # k=5, D->2D, GLU, residual, /sqrt2
  q  = h1 @ W_attn.T + b_attn
  A  = softmax(q @ enc.T) ; c = A @ (enc + femb)
  h2 = causal_cnn_glu(h1 + c, w1, b1)
  out = h2 @ W_proj.T + b_proj

Sharding: data-parallel over batch B=32 across 8 cores (4 batches/core),
weights replicated.  All activations on-chip are kept feature-major
([D partitions, T free]) so the causal conv taps are just shifted slices
along the free dim and matmul contractions stay on the partition dim.

Scale folding: the two /sqrt(2) are folded into the weights so the GLU
epilogue is exactly two DVE ops per half:
  x' = x/sqrt2  (W_lin,b_lin scaled), conv g-halves scaled by sqrt2,
  conv a-biases scaled by 1/sqrt2, attention context scaled by
  1/(sqrt2*denom) during normalization.
"""

import sys

try:  # prefer the environment's concourse (axon site); fall back to /opt
    import concourse  # noqa: F401
except ImportError:
    sys.path.insert(0, "/opt/trn_rl_repo")

from contextlib import ExitStack  # noqa: E402

import numpy as np  # noqa: E402

import concourse.bass as bass  # noqa: E402
import concourse.mybir as mybir  # noqa: E402
import concourse.tile as tile  # noqa: E402
from concourse import bacc  # noqa: E402
from concourse.masks import make_identity  # noqa: E402

F32 = mybir.dt.float32
F32R = mybir.dt.float32r
F16 = mybir.dt.float16
BF16 = mybir.dt.bfloat16
AF = mybir.ActivationFunctionType
OP = mybir.AluOpType

B, T_ENC, T_DEC, D, IN = 32, 1024, 2048, 256, 80
NCORES = 8
BPC = B // NCORES
SQRT2 = float(np.sqrt(2.0))
ISQ2 = float(1.0 / np.sqrt(2.0))
SHIFT = 50.0  # softmax stabilization: probs = exp(score - SHIFT)


def _r(ap):
    return ap.bitcast(F32R)


def build_nc(bpc=BPC, t_enc=T_ENC, t_dec=T_DEC, ch=512, num_devices=NCORES,
             loop_n=1, only_phase=None, pb_mode="pool", no_denom=False,
             rotate=False, tr_ident="f32", interleave=True, use_beta=True,
             et_pool="pmm", mel_split=4):
    nte = t_enc // 128   # encoder token tiles
    ntd = t_dec // 128   # decoder token tiles
    nch = t_dec // ch    # chunks per batch
    cpt = ch // 128      # 128-token tiles per chunk

    nc = bacc.Bacc("TRN2", target_bir_lowering=False, debug=False,
                   num_devices=num_devices)

    enc_d = nc.dram_tensor("enc", [bpc, t_enc, D], F16, kind="ExternalInput")
    femb_d = nc.dram_tensor("femb", [bpc, t_enc, D], F16, kind="ExternalInput")
    mel_d = nc.dram_tensor("mel", [bpc, t_dec, IN], F16, kind="ExternalInput")
    wlin_d = nc.dram_tensor("wlin", [IN, D], F16, kind="ExternalInput")
    w0_d = nc.dram_tensor("w0", [128, 5 * 2 * 2 * D], F16, kind="ExternalInput")
    w1_d = nc.dram_tensor("w1", [128, 5 * 2 * 2 * D], F16, kind="ExternalInput")
    wattn_d = nc.dram_tensor("wattn", [128, 2 * D], F16, kind="ExternalInput")
    wproj_d = nc.dram_tensor("wproj", [128, 2 * IN], F16, kind="ExternalInput")
    bias_d = nc.dram_tensor("bias", [128, 13], F32, kind="ExternalInput")
    ident_d = nc.dram_tensor("ident", [128, 128], F32, kind="ExternalInput")
    out_d = nc.dram_tensor("out", [bpc, t_dec, IN], F32, kind="ExternalOutput")

    with tile.TileContext(nc) as tc, ExitStack() as ctx:
        cpool = ctx.enter_context(tc.tile_pool(name="const", bufs=1))
        stage = ctx.enter_context(tc.tile_pool(name="stage", bufs=4))
        pb = ctx.enter_context(tc.tile_pool(name="perbatch", bufs=1))
        sc = ctx.enter_context(tc.tile_pool(name="scratch", bufs=1))
        sc2 = ctx.enter_context(tc.tile_pool(name="scratch2", bufs=2))
        pmm = ctx.enter_context(
            tc.tile_pool(name="pmm", bufs=6, space=bass.MemorySpace.PSUM))
        pctx = ctx.enter_context(
            tc.tile_pool(name="pctx", bufs=1, space=bass.MemorySpace.PSUM))

        # ---- constants ----
        ident = cpool.tile([128, 128], F32, tag="ident")
        nc.sync.dma_start(out=ident[:], in_=ident_d[:])
        # fp16 transposes measured 41.8ns vs f32's 140ns per [128,128] tile;
        # the permutation matrix is exact in fp16 and the data is fp16 anyway
        identT = cpool.tile([128, 128], F16, tag="identT")
        nc.vector.tensor_copy(identT[:], ident[:])
        # ones column scaled by sqrt2: denominator comes out as sqrt2*sum(p),
        # so reciprocal directly gives isq2/sum(p) (folds the GLU 1/sqrt2).
        # (DVE memset can't write f32r; stage in f32 and copy with rounding.)
        ones_f32 = cpool.tile([128, 1], F32, tag="ones_f32")
        nc.vector.memset(ones_f32[:], SQRT2)
        ones_col = cpool.tile([128, 1], F32R, tag="ones")
        nc.vector.tensor_copy(ones_col[:], ones_f32[:])
        onesr_f32 = cpool.tile([1, 128], F32, tag="onesr_f32")
        nc.vector.memset(onesr_f32[:], 1.0)
        ones_row = cpool.tile([1, 128], F32R, tag="ones_row")
        nc.vector.tensor_copy(ones_row[:], onesr_f32[:])
        zero4 = cpool.tile([128, 4], F32, tag="zero4")
        nc.vector.memset(zero4[:], 0.0)
        negshift = cpool.tile([128, 1], F32, tag="negshift")
        nc.vector.memset(negshift[:], -SHIFT)

        wlin = cpool.tile([IN, D], F16, tag="wlin")
        w0 = cpool.tile([128, 5 * 2 * 2 * D], F16, tag="w0")
        w1 = cpool.tile([128, 5 * 2 * 2 * D], F16, tag="w1")
        wattn = cpool.tile([128, 2 * D], F16, tag="wattn")
        wproj = cpool.tile([128, 2 * IN], F16, tag="wproj")
        bias = cpool.tile([128, 13], F32, tag="bias")

        def load_weights_early():
            nc.sync.dma_start(out=wlin[:], in_=wlin_d[:])
            nc.sync.dma_start(out=bias[:], in_=bias_d[:])

        def load_weights_mid():
            nc.sync.dma_start(out=w0[:], in_=w0_d[:])

        def load_weights_late():
            nc.sync.dma_start(out=wattn[:], in_=wattn_d[:])
            nc.sync.dma_start(out=wproj[:], in_=wproj_d[:])
            nc.sync.dma_start(out=w1[:], in_=w1_d[:])

        def bcol(j):
            return bias[:, j:j + 1]

        def tr(out_ap, in_ap, id_ap):
            nc.tensor.transpose(out_ap, in_ap, id_ap)

        def conv_glu(w_sb, ba0, bg0, in_buf, base, out_ap_fn, resid_ap_fn):
            """One causal-conv+GLU chunk.  in_buf: [128, 2, T+4] padded buffer.
            out_ap_fn(i) / resid_ap_fn(i) give [128, ch] APs for d-tile i."""
            s_tiles = {}
            for j in (2, 3, 0, 1):
                pc = pmm.tile([128, ch], F32, tag="mm")
                k = 0
                for t in range(5):
                    for i in range(2):
                        col = (t * 2 + i) * (2 * D) + j * 128
                        nc.tensor.matmul(
                            pc[:],
                            w_sb[:, col:col + 128],
                            in_buf[:, i, base + t:base + t + ch],
                            start=(k == 0), stop=(k == 9))
                        k += 1
                if j >= 2:
                    s = sc2.tile([128, ch], F32, tag=f"sig{j - 2}", name=f"sig{j - 2}")
                    nc.scalar.activation(s[:], pc[:], AF.Sigmoid,
                                         bias=bcol(bg0 + (j - 2)))
                    s_tiles[j - 2] = s
                else:
                    o = out_ap_fn(j)
                    nc.vector.scalar_tensor_tensor(
                        o, pc[:], bcol(ba0 + j), s_tiles[j][:],
                        op0=OP.add, op1=OP.mult)
                    nc.vector.tensor_add(o, o, resid_ap_fn(j))

        def prep_mel(b, melT, x_buf):
            """mel load (one DMA), transpose to feature-major, zero pads."""
            with nc.named_scope(f"prepmel{b}"):
                if b == 0:
                    load_weights_early()
                nc.vector.tensor_copy(melT[:, 0:1], zero4[0:IN, 0:1])
                for i in range(2):
                    nc.vector.tensor_copy(x_buf[:, i, 0:4], zero4[:])
                mt = stage.tile([128, ntd, IN], F16, tag="mt", name="mt", bufs=1)
                mview = mel_d[b].rearrange("(n p) f -> p n f", p=128)
                # quarter the load so the first transpose starts ~4x earlier
                nq = ntd // mel_split
                for qq in range(mel_split):
                    nc.sync.dma_start(out=mt[:, qq * nq:(qq + 1) * nq, :],
                                      in_=mview[:, qq * nq:(qq + 1) * nq, :])
                for n in range(ntd):
                    pt = pmm.tile([IN, 128], F16, tag="mm", name="pt")
                    tr(pt[:], mt[:, n, :], identT[:])
                    dst = melT[:, 1 + n * 128:1 + (n + 1) * 128]
                    if n % 2 == 0:
                        nc.vector.tensor_copy(dst, pt[:])
                    else:
                        nc.scalar.copy(dst, pt[:])
                if b == 0:
                    load_weights_mid()

        def prep_enc(b, encT, encsum, eT, bias_phB, battn_r):
            # enc/femb are loaded with partition-outer token order (contiguous
            # 8KB runs per partition, ~8x fewer DMA descriptors).  This
            # permutes encoder positions, which cancels: encT (scores
            # stationary) and encsum (context stationary) use the identical
            # permutation and attention is permutation-invariant over ts.
            with nc.named_scope(f"prepenc{b}"):
                et = stage.tile([128, nte, D], F16, tag="et", name="et", bufs=1)
                nc.sync.dma_start(
                    out=et[:], in_=enc_d[b].rearrange("(p n) d -> p n d", p=128))
                for n in range(nte):
                    for i in range(2):
                        pt = pmm.tile([128, 128], F16, tag="mm", name="pt")
                        tr(pt[:], et[:, n, i * 128:(i + 1) * 128], identT[:])
                        nc.scalar.copy(encT[:, i, n * 128:(n + 1) * 128], pt[:])
                if b == 0:
                    load_weights_late()
                    for i2 in range(2):
                        nc.vector.tensor_copy(battn_r[:, i2, 0:1],
                                              bias[:, 10 + i2:11 + i2])
                        nc.vector.tensor_copy(battn_r[:, i2, 1:2],
                                              zero4[:, 0:1])
                # ẽ = W_attn^T·enc in feature-major [d̃, s]: replaces the
                # per-chunk q projection (1024 enc rows vs 2048 dec rows).
                S = min(ch, 512)
                for o in range(2):
                    for sh in range(t_enc // S):
                        pe_t = (pmm if et_pool == "pmm" else pctx).tile(
                            [128, ch], F32, tag="mm" if et_pool == "pmm" else "c0",
                            name="pe_t")
                        for i in range(2):
                            col = (i * 2 + o) * 128
                            nc.tensor.matmul(
                                pe_t[:, 0:S],
                                wattn[:, col:col + 128],
                                encT[:, i, sh * S:(sh + 1) * S],
                                start=(i == 0), stop=(i == 1))
                        nc.scalar.copy(eT[:, o, sh * S:(sh + 1) * S],
                                       pe_t[:, 0:S])
                # β_j[s] = b_attn·enc_s folded into the exp bias (with -SHIFT)
                if use_beta:
                    pbeta = (pmm if et_pool == "pmm" else pctx).tile(
                        [128, ch], F32, tag="mm" if et_pool == "pmm" else "c1",
                        name="pbeta")
                    for j in range(nte):
                        for i in range(2):
                            nc.tensor.matmul(pbeta[:, 2 * j:2 * j + 2],
                                             encT[:, i, j * 128:(j + 1) * 128],
                                             battn_r[:, i, :],
                                             start=(i == 0), stop=(i == 1))
                    nc.scalar.activation(
                        bias_phB[:],
                        pbeta[:].rearrange("p (h two) -> p h two", two=2)[:, 0:nte, 0],
                        AF.Identity, bias=negshift[:])
                fview = femb_d[b].rearrange("(p n) d -> p n d", p=128)
                for h in range(2):
                    nh = nte // 2
                    ft = stage.tile([128, nh, D], F16, tag="ft", name="ft",
                                    bufs=2)
                    nc.sync.dma_start(out=ft[:], in_=fview[:, h * nh:(h + 1) * nh, :])
                    for n in range(nh):
                        nc.vector.tensor_add(encsum[:, h * nh + n, :],
                                             et[:, h * nh + n, :], ft[:, n, :])

        def body_emit():
            melTs, x_bufs, h1_bufs = {}, {}, {}

            def alloc_mel(b):
                melTs[b] = pb.tile([IN, t_dec + 1], F16, tag="melT", name="melT")
                x_bufs[b] = pb.tile([128, 2, t_dec + 4], F16, tag="x_buf",
                                    name="x_buf")

            def emit_phA(b):
                """linear + conv0 for all chunks of batch b (sigmoid table)."""
                melT = melTs.pop(b)
                x_buf = x_bufs[b]
                h1_bufs[b] = pb.tile([128, 2, t_dec], F16, tag="h1_buf",
                                     name="h1_buf")
                h1_buf = h1_bufs[b]
                with nc.named_scope(f"phA_{b}"):
                    for c in range(nch):
                        base = c * ch
                        for i in range(2):
                            px = pmm.tile([128, ch], F32, tag="mm", name="px")
                            nc.tensor.matmul(px[:],
                                             wlin[:, i * 128:(i + 1) * 128],
                                             melT[:, base:base + ch],
                                             start=True, stop=True)
                            nc.scalar.activation(
                                x_buf[:, i, 4 + base:4 + base + ch],
                                px[:], AF.Identity, bias=bcol(0 + i))
                        if c == 0:
                            # x[0] must be exactly 0 (shift pad), not b_lin
                            for i2 in range(2):
                                nc.vector.tensor_copy(x_buf[:, i2, 4:5],
                                                      zero4[:, 0:1])
                    for c in range(nch):
                        base = c * ch
                        conv_glu(w0, 2, 4, x_buf, base,
                                 lambda i: h1_buf[:, i, base:base + ch],
                                 lambda i: x_buf[:, i, 4 + base:4 + base + ch])

            if only_phase in ("B", "C"):
                # isolated-phase builds still need weights + dummy producers
                load_weights_early()
                load_weights_mid()
                load_weights_late()
            alloc_mel(0)
            if only_phase in (None, "A"):
                prep_mel(0, melTs[0], x_bufs[0])
                emit_phA(0)

            battn_r = cpool.tile([128, 2, 2], F16, tag="battn_r")
            for b in range(bpc):
                if not rotate and b > 0 and only_phase in (None, "A"):
                    emit_phA(b)
                encT = pb.tile([128, 2, t_enc], F16, tag="encT", name="encT")
                encsum = pb.tile([128, nte, D], F32R, tag="encsum", name="encsum")
                eT = pb.tile([128, 2, t_enc], F16, tag="eT", name="eT")
                bias_phB = pb.tile([128, nte], F32, tag="bias_phB",
                                   name="bias_phB")
                hA_buf = pb.tile([128, 2, t_dec + 4], F16, tag="hA_buf",
                                 name="hA_buf")
                if only_phase == "B":
                    h1_bufs[b] = pb.tile([128, 2, t_dec], F16, tag="h1_buf",
                                         name="h1_buf")
                h1_buf = h1_bufs.get(b)

                if only_phase in (None, "B"):
                    prep_enc(b, encT, encsum, eT, bias_phB, battn_r)
                # next batch's mel prep hides under phB/phC of this batch
                if b + 1 < bpc:
                    alloc_mel(b + 1)
                    if only_phase in (None, "A"):
                        prep_mel(b + 1, melTs[b + 1], x_bufs[b + 1])

                # hA zero pads (hA_buf slot frees once conv1 of b-1 is done)
                for i in range(2):
                    nc.vector.tensor_copy(hA_buf[:, i, 0:4], zero4[:])
                if only_phase == "B":
                    for i in range(2):
                        nc.vector.tensor_copy(h1_buf[:, i, 0:4], zero4[:])
                if only_phase == "C":
                    for i in range(2):
                        nc.vector.tensor_copy(hA_buf[:, i, 4:8], zero4[:])

                # ---- phase B: attention for all chunks (exp table) ----
                def scores_exp(c):
                    base = c * ch
                    probs = sc.tile([128, nte, ch], F32R, tag="probs",
                                    name="probs", bufs=2)
                    for j in range(nte):
                        ps = pmm.tile([128, ch], F32, tag="mm", name="ps")
                        for i in range(2):
                            nc.tensor.matmul(ps[:],
                                             eT[:, i, j * 128:(j + 1) * 128],
                                             h1_buf[:, i, base:base + ch],
                                             start=(i == 0), stop=(i == 1))
                        nc.scalar.activation(
                            probs[:, j, :], ps[:], AF.Exp,
                            bias=(bias_phB[:, j:j + 1] if use_beta
                                  else negshift[:]))
                    return probs

                def attn_back(c, probs):
                    base = c * ch
                    pc0 = pctx.tile([128, ch], F32, tag="c0", name="pc0")
                    pc1 = pctx.tile([128, ch], F32, tag="c1", name="pc1")
                    pd = pmm.tile([1, ch], F32, tag="mm", name="pd")
                    for j in range(nte):
                        pr = probs[:, j, :]
                        nc.tensor.matmul(pc0[:], encsum[:, j, 0:128], pr,
                                         start=(j == 0), stop=(j == nte - 1))
                        nc.tensor.matmul(pc1[:], encsum[:, j, 128:256], pr,
                                         start=(j == 0), stop=(j == nte - 1))
                        if not no_denom:
                            nc.tensor.matmul(pd[:], ones_col[:], pr,
                                             start=(j == 0), stop=(j == nte - 1))
                    den_r = sc.tile([1, ch], F32, tag="den", name="den")
                    rep = sc.tile([128, ch], F32, tag="rep", name="rep")
                    if no_denom:
                        nc.vector.memset(rep[:], 1.0)
                    elif pb_mode == "pool":
                        nc.vector.reciprocal(den_r[:], pd[:])
                        nc.gpsimd.partition_broadcast(rep[:], den_r[:])
                    else:
                        nc.vector.reciprocal(den_r[:], pd[:])
                        den_rr = sc.tile([1, ch], F32R, tag="denr", name="den_rr")
                        nc.vector.tensor_copy(den_rr[:], den_r[:])
                        prep_ps = pmm.tile([128, ch], F32, tag="mm", name="prep_ps")
                        nc.tensor.matmul(prep_ps[:], ones_row[:], den_rr[:],
                                         start=True, stop=True)
                        nc.vector.tensor_copy(rep[:], prep_ps[:])
                    pcx = [pc0, pc1]
                    for i in range(2):
                        tmp = sc.tile([128, ch], F32, tag=f"tmp{i}", name=f"tmp{i}")
                        nc.vector.tensor_tensor(tmp[:], pcx[i][:], rep[:], op=OP.mult)
                        # hA' = h1/sqrt2 + ctx_unnorm * (isq2/denom)
                        nc.vector.scalar_tensor_tensor(
                            hA_buf[:, i, 4 + base:4 + base + ch],
                            h1_buf[:, i, base:base + ch], ISQ2, tmp[:],
                            op0=OP.mult, op1=OP.add)

                if only_phase in (None, "B"):
                    with nc.named_scope(f"phB_{b}"):
                        if interleave:
                            # scores(c+1) before ctx(c): PE chews on the next
                            # chunk's scores while ACT evaluates exp(c)
                            probs_c = scores_exp(0)
                            for c in range(nch):
                                probs_next = (scores_exp(c + 1)
                                              if c + 1 < nch else None)
                                attn_back(c, probs_c)
                                probs_c = probs_next
                        else:
                            for c in range(nch):
                                attn_back(c, scores_exp(c))

                # rotated schedule: conv0 of batch b+1 is emitted here, between
                # phB(b) and phC(b).  phC(b) depends on phB(b)'s DVE epilogue
                # (hA), so conv0(b+1) gives PE independent work to chew on while
                # that drains; ACT table order stays exp -> sigmoid -> sigmoid.
                if rotate and b + 1 < bpc and only_phase in (None, "A"):
                    emit_phA(b + 1)

                # ---- phase C: conv1 + proj for all chunks (sigmoid table) ----
                # proj(c-1) is emitted after conv1(c) so PE never waits on the
                # GLU DVE epilogue of chunk c before starting useful work.
                def proj_and_out(c, h2, last=False):
                    base = c * ch
                    pp = pmm.tile([IN, ch], F32, tag="mm", name="pp")
                    for kk in range(2):
                        nc.tensor.matmul(pp[:], wproj[:, kk * IN:(kk + 1) * IN],
                                         h2[kk][:], start=(kk == 0),
                                         stop=(kk == 1))
                    proj = sc2.tile([IN, ch], F16, tag="proj", name="proj")
                    nc.scalar.activation(proj[:], pp[:], AF.Identity,
                                         bias=bias[0:IN, 12:13])
                    for k in range(cpt):
                        if last:
                            # conv is finished: use the big pool so all four
                            # transposes fly in parallel during the drain
                            pt = pmm.tile([128, IN], F16, tag="mm", name="pt")
                        else:
                            pt = pctx.tile([128, IN], F16, tag=("c0" if k % 2 == 0
                                                               else "c1"), name="pt")
                        tr(pt[:], proj[:, k * 128:(k + 1) * 128], identT[0:IN, 0:IN])
                        ot = sc2.tile([128, IN], F32, tag="outT", name="ot")
                        if k % 2 == 0:
                            nc.vector.tensor_copy(ot[:], pt[:])
                        else:
                            nc.scalar.copy(ot[:], pt[:])
                        r0 = base + k * 128
                        # alternate HWDGE queues (SP / Activation) so the
                        # descriptor processing of consecutive tiles overlaps
                        eng = nc.sync if k % 2 == 0 else nc.scalar
                        eng.dma_start(out=out_d[b, r0:r0 + 128, :], in_=ot[:])

                if only_phase in (None, "C"):
                    with nc.named_scope(f"phC_{b}"):
                        h2_prev = None
                        for c in range(nch):
                            base = c * ch
                            h2 = [sc2.tile([128, ch], F16, tag=f"h2_{i}",
                                           name=f"h2_{i}") for i in range(2)]
                            conv_glu(w1, 6, 8, hA_buf, base,
                                     lambda i: h2[i][:],
                                     lambda i: hA_buf[:, i, 4 + base:4 + base + ch])
                            if h2_prev is not None:
                                proj_and_out(c - 1, h2_prev)
                            h2_prev = h2
                        proj_and_out(nch - 1, h2_prev, last=(b == bpc - 1))
        import contextlib
        loop_cm = (tc.For_i(0, loop_n, 1, hint_engines=(mybir.EngineType.PE,))
                   if loop_n > 1 else contextlib.nullcontext())
        with loop_cm:
            body_emit()

    nc.compile()
    return nc


def prep_weights(W_lin, b_lin, conv_w0, conv_b0, conv_w1, conv_b1,
                 W_attn, b_attn, W_proj, b_proj):
    def prep_conv(w):
        ws = w.astype(np.float32).copy()
        ws[D:] *= SQRT2                       # g-half
        # [512, 256, 5] -> [p, t, i, o] -> [128, 5*2*512]
        arr = ws.transpose(1, 2, 0).reshape(2, 128, 5, 2 * D).transpose(1, 2, 0, 3)
        return np.ascontiguousarray(arr.reshape(128, 5 * 2 * 2 * D)
                                    ).astype(np.float16)

    wlin_h = np.ascontiguousarray(W_lin.T * ISQ2).astype(np.float16)
    # lhsT layout for ẽ = W_attn^T·enc: [e_part, (i,o), d̃] where i is the
    # e-contraction tile and o the output d̃-tile:
    # wattn_h[p, (i*2+o)*128 + c] = W_attn[i*128+p, o*128+c]
    wattn_h = np.ascontiguousarray(
        np.asarray(W_attn, np.float32).reshape(2, 128, 2, 128)
        .transpose(1, 0, 2, 3).reshape(128, 2 * D)).astype(np.float16)
    wproj_h = np.ascontiguousarray(
        W_proj.T.reshape(2, 128, IN).transpose(1, 0, 2).reshape(128, 2 * IN)
    ).astype(np.float16)

    bias_h = np.zeros((128, 13), np.float32)
    bias_h[:, 0] = b_lin[0:128] * ISQ2
    bias_h[:, 1] = b_lin[128:256] * ISQ2
    bias_h[:, 2] = conv_b0[0:128] * ISQ2      # a-half biases scaled
    bias_h[:, 3] = conv_b0[128:256] * ISQ2
    bias_h[:, 4] = conv_b0[256:384]           # g-half biases unscaled
    bias_h[:, 5] = conv_b0[384:512]
    bias_h[:, 6] = conv_b1[0:128] * ISQ2
    bias_h[:, 7] = conv_b1[128:256] * ISQ2
    bias_h[:, 8] = conv_b1[256:384]
    bias_h[:, 9] = conv_b1[384:512]
    bias_h[:, 10] = b_attn[0:128]
    bias_h[:, 11] = b_attn[128:256]
    bias_h[0:IN, 12] = b_proj

    return {
        "wlin": wlin_h, "w0": prep_conv(conv_w0), "w1": prep_conv(conv_w1),
        "wattn": wattn_h, "wproj": wproj_h, "bias": bias_h,
        "ident": np.eye(128, dtype=np.float32),
    }


def make_in_maps(inputs, w=None):
    """Shard FULL inputs into per-core in_maps (fp16 activations + weights)."""
    if w is None:
        w = prep_weights(*[inputs[n] for n in (
            "W_lin", "b_lin", "conv_w0", "conv_b0", "conv_w1", "conv_b1",
            "W_attn", "b_attn", "W_proj", "b_proj")])
    enc = np.asarray(inputs["encoder_outputs"], np.float32).astype(np.float16)
    femb = np.asarray(inputs["first_embedding"], np.float32).astype(np.float16)
    mel = np.asarray(inputs["mel_inputs"], np.float32).astype(np.float16)
    in_maps = []
    for c in range(NCORES):
        sl = slice(c * BPC, (c + 1) * BPC)
        in_maps.append({"enc": np.ascontiguousarray(enc[sl]),
                        "femb": np.ascontiguousarray(femb[sl]),
                        "mel": np.ascontiguousarray(mel[sl]), **w})
    return in_maps


_NC = None


def _get_nc():
    global _NC
    if _NC is None:
        _NC = build_nc()
    return _NC


def kernel(encoder_outputs, first_embedding, mel_inputs,
           W_lin, b_lin, conv_w0, conv_b0, conv_w1, conv_b1,
           W_attn, b_attn, W_proj, b_proj):
    from concourse.bass_utils import run_bass_kernel_spmd

    nc = _get_nc()
    w = prep_weights(W_lin, b_lin, conv_w0, conv_b0, conv_w1, conv_b1,
                     W_attn, b_attn, W_proj, b_proj)
    in_maps = make_in_maps(
        {"encoder_outputs": encoder_outputs, "first_embedding": first_embedding,
         "mel_inputs": mel_inputs}, w=w)
    res = run_bass_kernel_spmd(nc, in_maps, list(range(NCORES)))
    return np.concatenate([res.results[i]["out"] for i in range(NCORES)], axis=0)

